# revision 15
# baseline (speedup 1.0000x reference)
"""NLL sequence loss kernel for Trainium2 (8 NeuronCores, SPMD batch-parallel).

Reference semantics (B=512, T=128, C=2000):
    last[b] = min(T, length[b]) - 1
    out = sum_b(-inputs[b, last[b], target[b]] * (length[b] >= 1)) / sum_b(length[b] >= 1)

Only one element per batch row is ever read, so instead of streaming the
full 512 MB input, each core keeps its 64 MB batch shard in HBM and does a
64-element indirect-DMA gather at device-computed flat offsets. The offset
list lives one-per-partition ([64, 1]) — the HW DGE reads it that way.

Raw Bass (no Tile): the whole kernel is a short serial chain
    meta DMA -> index math (DVE) -> indirect gather (Pool SWDGE)
             -> mask (DVE) -> cross-partition reduce (PE matmul w/ ones)
             -> PSUM copy (DVE) -> out DMA
with explicit single-wait semaphores (the TPB ISA has one wait slot per
instruction, which Tile's auto-generated tail drain exceeds).
"""

import numpy as np

import concourse.bass as bass
import concourse.mybir as mybir
from concourse.bass_utils import run_bass_kernel_spmd

B, T, C = 512, 128, 2000
N_CORES = 8
BS = B // N_CORES  # 64 batch rows per core
N = BS * T * C     # flat elements per shard


def build_nc() -> bass.Bass:
    nc = bass.Bass()
    x = nc.declare_dram_parameter("x", [N, 1], mybir.dt.float32, isOutput=False)
    # meta[b] = [length[b], target[b], b*T*C] -> one DMA, one row per partition
    meta = nc.declare_dram_parameter("meta", [BS, 3], mybir.dt.int32, isOutput=False)
    out = nc.declare_dram_parameter("out", [2], mybir.dt.float32, isOutput=True)

    Alu = mybir.AluOpType
    with (
        nc.sbuf_tensor([BS, 3], mybir.dt.int32) as meta_t,
        nc.sbuf_tensor([BS, 1], mybir.dt.int32) as idx_t,
        nc.sbuf_tensor([BS, 1], mybir.dt.float32) as vals_t,
        nc.sbuf_tensor([BS, 2], mybir.dt.float32) as stack_t,  # [masked | valid]
        nc.sbuf_tensor([BS, 1], mybir.dt.float32) as ones_t,
        nc.sbuf_tensor([1, 2], mybir.dt.float32) as red_t,
        nc.psum_tensor([1, 2], mybir.dt.float32) as psum_t,
        nc.semaphore() as dsem,   # SP HWDGE completions (load, then store)
        nc.semaphore() as vsem,   # DVE progress
        nc.semaphore() as gsem,   # gather completion
        nc.semaphore() as psem,   # PE matmul done
        nc.Block() as block,
    ):
        len_ap = meta_t[:, 0:1]
        tgt_ap = meta_t[:, 1:2]
        iot_ap = meta_t[:, 2:3]
        masked_ap = stack_t[:, 0:1]
        valid_ap = stack_t[:, 1:2]

        @block.sync
        def _(sync):
            sync.dma_start(out=meta_t[:, :], in_=meta[:, :]).then_inc(dsem, 16)
            sync.wait_ge(vsem, 7)
            sync.dma_start(out=out[:], in_=red_t[:1, :2]).then_inc(dsem, 16)
            sync.wait_ge(dsem, 32)

        @block.vector
        def _(vector):
            # engines are pipelined: every same-engine RAW needs its own
            # inc/wait pair (the race detector flags bare back-to-back RAW)
            nc.vector.memset(ones_t[:, :1], 1.0)  # no deps
            vector.wait_ge(dsem, 16)
            # idx = max(min(len, T) - 1, 0) * C + tgt + b*T*C
            nc.vector.tensor_scalar(
                out=idx_t[:, :1], in0=len_ap,
                scalar1=T, scalar2=-1, op0=Alu.min, op1=Alu.add,
            ).then_inc(vsem, 1)
            vector.wait_ge(vsem, 1)
            nc.vector.tensor_scalar(
                out=idx_t[:, :1], in0=idx_t[:, :1],
                scalar1=0, scalar2=C, op0=Alu.max, op1=Alu.mult,
            ).then_inc(vsem, 1)
            vector.wait_ge(vsem, 2)
            nc.vector.tensor_tensor(
                out=idx_t[:, :1], in0=idx_t[:, :1], in1=tgt_ap, op=Alu.add
            ).then_inc(vsem, 1)
            vector.wait_ge(vsem, 3)
            nc.vector.tensor_tensor(
                out=idx_t[:, :1], in0=idx_t[:, :1], in1=iot_ap, op=Alu.add
            ).then_inc(vsem, 1)  # vsem=4: gather may start
            # off the gather critical path: valid mask (length >= 1) as f32
            nc.vector.tensor_scalar(
                out=valid_ap, in0=len_ap,
                scalar1=1, scalar2=None, op0=Alu.is_ge,
            ).then_inc(vsem, 1)  # vsem=5
            vector.wait_ge(gsem, 16)
            vector.wait_ge(vsem, 5)
            nc.vector.tensor_tensor(
                out=masked_ap, in0=vals_t[:, :1], in1=valid_ap, op=Alu.mult
            ).then_inc(vsem, 1)  # vsem=6: matmul may start
            vector.wait_ge(psem, 1)
            nc.vector.tensor_copy(
                out=red_t[:1, :2], in_=psum_t[:1, :2]
            ).then_inc(vsem, 1)  # vsem=7: out store may start

        @block.gpsimd
        def _(gpsimd):
            gpsimd.wait_ge(vsem, 4)
            nc.gpsimd.indirect_dma_start(
                out=vals_t[:, :1],
                out_offset=None,
                in_=x[:, :],
                in_offset=bass.IndirectOffsetOnAxis(ap=idx_t[:, :1], axis=0),
            ).then_inc(gsem, 16)

        @block.tensor
        def _(tensor):
            tensor.wait_ge(vsem, 6)
            # [1,2] = ones[64,1].T @ [masked | valid][64,2]
            nc.tensor.matmul(
                out=psum_t[:1, :2],
                lhsT=ones_t[:, :1],
                rhs=stack_t[:, :2],
                start=True,
                stop=True,
            ).then_inc(psem, 1)

    return nc


_IOTA = (np.arange(BS, dtype=np.int64) * T * C).astype(np.int32)


def run(inputs, length, target, **spmd_kwargs):
    """Shard, run on 8 cores, combine. Returns (scalar result, BassKernelResults)."""
    x = np.ascontiguousarray(np.asarray(inputs, dtype=np.float32))
    ln = np.ascontiguousarray(np.asarray(length).astype(np.int32))
    tg = np.ascontiguousarray(np.asarray(target).astype(np.int32))
    assert x.shape == (B, T, C), x.shape

    nc = build_nc()
    in_maps = []
    for c in range(N_CORES):
        sl = slice(c * BS, (c + 1) * BS)
        in_maps.append(
            {
                "x": x[sl].reshape(N, 1),
                "meta": np.ascontiguousarray(
                    np.stack([ln[sl], tg[sl], _IOTA], axis=1)
                ),
            }
        )
    r = run_bass_kernel_spmd(nc, in_maps, list(range(N_CORES)), **spmd_kwargs)
    num = sum(float(m["out"][0]) for m in r.results)
    cnt = sum(float(m["out"][1]) for m in r.results)
    return np.asarray(np.float32(-num / cnt)), r


def kernel(**inputs: np.ndarray) -> np.ndarray:
    return run(inputs["inputs"], inputs["length"], inputs["target"])[0]
